# revision 1
# baseline (speedup 1.0000x reference)
"""3-layer GCN (message passing) on 8 TRN2 NeuronCores.

Strategy: shard destination nodes across cores (graph parallel). Each layer:
  h'_T = dinv * (prev @ W)  computed locally on the node shard (PE),
  AllGather h' rows (bf16) so every core sees all source features,
  per group of 4 dst blocks: dma_gather source rows on 4 parallel SWDGE
  queues (edges sorted by dst block, lo/hi split so gather indices fit
  int16), build the weighted one-hot S_w[e,d] = (dstloc[e]==d)*w[e] for the
  whole group in 2 broadcast-AP DVE ops, and accumulate out_T = M.T @ S_w
  on the PE into a group-wide PSUM bank. Epilogue (batched per group):
  dinv scale + Prelu(x+bias, 0.1).
The degree pass reuses the same S_w tiles with a ones-vector matmul.
"""

import numpy as np

import concourse.bacc as bacc
import concourse.mybir as mybir
from concourse.tile import TileContext
from concourse.bass_utils import run_bass_kernel_spmd

try:
    import ml_dtypes

    BF16 = ml_dtypes.bfloat16
except ImportError:  # pragma: no cover
    BF16 = None

N_CORES = 8
D = 128
NEG_SLOPE = 0.1
G_BLOCKS = 1  # dst blocks per gather call group
LEAKY_VIA_PRELU = True  # sim validation sets False (Prelu not in CoreSim)


def _ceil_div(a, b):
    return (a + b - 1) // b


def _wrap_idx(idx):
    """[cnt] int16 -> [128, cnt//16] wrapped layout (16-partition, replicated x8)."""
    cnt = idx.shape[0]
    assert cnt % 16 == 0
    w = idx.reshape(cnt // 16, 16).T  # [16, cnt//16]
    return np.tile(w, (8, 1)).astype(np.int16)  # [128, cnt//16]


def _preprocess(x, edge_index, edge_attr, edge_type):
    """Host-side sharding/layout. Returns (meta, per-core input arrays)."""
    N = x.shape[0]
    E = edge_index.shape[1]
    assert N % N_CORES == 0
    per = N // N_CORES
    nb = _ceil_div(per, 128)
    per_pad = nb * 128
    # split each core's shard rows at SA: half A rows [0,SA), half B [SA,per).
    # Gather tables are the two AllGather outputs (rank-major concat).
    SA = max(16, ((per // 2) // 16) * 16)
    SB = per - SA
    assert SA * N_CORES <= 32767 + 1 and SB * N_CORES <= 32767 + 1

    # self-loops are NOT materialized as edges: the epilogue adds h'[d]
    # directly and the degree pass adds the +1 analytically.
    src_f = np.asarray(edge_index[0], dtype=np.int64)
    dst_f = np.asarray(edge_index[1], dtype=np.int64)
    attr_f = np.asarray(edge_attr, dtype=np.float32)
    type_f = np.asarray(edge_type, dtype=np.float32)

    core = dst_f // per
    ldst = dst_f - core * per
    blk = ldst >> 7
    slot = ldst & 127
    src_c = src_f // per
    src_r = src_f - src_c * per
    half = (src_r >= SA).astype(np.int64)
    # gather index within the half table (rank-major AllGather layout)
    gidx = np.where(half == 0, src_c * SA + src_r, src_c * SB + (src_r - SA))

    counts = np.zeros((N_CORES, nb, 2), dtype=np.int64)
    per_core = []
    for c in range(N_CORES):
        m = core == c
        s_src = src_f[m]
        s_blk = blk[m]
        s_half = half[m]
        order = np.lexsort((s_src, s_half, s_blk))
        per_core.append(
            dict(
                src=gidx[m][order],
                half=s_half[order],
                blk=s_blk[order],
                slot=slot[m][order],
                attr=attr_f[m][order],
                type=type_f[m][order],
            )
        )
        cnt = np.bincount(s_blk * 2 + s_half, minlength=nb * 2).reshape(nb, 2)
        counts[c] = cnt

    # common padded schedule: tiles per (block, half), maxed over cores
    tiles_bh = np.maximum(1, _ceil_div(counts.max(axis=0), 128))  # [nb, 2]
    pad_bh = tiles_bh * 128

    groups = [list(range(g, min(g + G_BLOCKS, nb))) for g in range(0, nb, G_BLOCKS)]
    slot_off = np.zeros((nb, 2), dtype=np.int64)
    call_cnt = []  # per (g, half): total padded count
    off = 0
    for g in groups:
        for h in (0, 1):
            c0 = off
            for b in g:
                slot_off[b, h] = off
                off += pad_bh[b, h]
            call_cnt.append(off - c0)
    totslot = off
    T = totslot // 128

    tcols_b = []
    for b in range(nb):
        cols = list(range(slot_off[b, 0] // 128, slot_off[b, 0] // 128 + tiles_bh[b, 0]))
        cols += list(range(slot_off[b, 1] // 128, slot_off[b, 1] // 128 + tiles_bh[b, 1]))
        tcols_b.append(cols)

    ins = []
    for c in range(N_CORES):
        pc = per_core[c]
        idx_sl = np.zeros(totslot, dtype=np.int16)
        dst_sl = np.zeros(totslot, dtype=np.float32)
        at_sl = np.zeros(totslot, dtype=np.float32)
        ty_sl = np.zeros(totslot, dtype=np.float32)
        e0 = 0
        for b in range(nb):
            for h in (0, 1):
                n = counts[c, b, h]
                o = slot_off[b, h]
                if n:
                    sl = slice(e0, e0 + n)
                    idx_sl[o : o + n] = pc["src"][sl].astype(np.int16)
                    dst_sl[o : o + n] = pc["slot"][sl]
                    at_sl[o : o + n] = pc["attr"][sl]
                    ty_sl[o : o + n] = pc["type"][sl]
                    e0 += n

        wrapped = []
        off2 = 0
        for cc in call_cnt:
            wrapped.append(_wrap_idx(idx_sl[off2 : off2 + cc]))
            off2 += cc
        idx_w = np.concatenate(wrapped, axis=1)  # [128, totslot//16]

        col = lambda a: np.ascontiguousarray(a.reshape(T, 128).T)  # [128, T]
        dst_col = np.ascontiguousarray(dst_sl.reshape(T, 128).T).astype(BF16)
        sraw = np.zeros((totslot, 128), dtype=BF16)
        sraw[np.arange(totslot), dst_sl.astype(np.int64)] = 1.0
        # pad slots (attr==0) -> weight 0 anyway; keep their one-hot harmless
        xt = np.zeros((128, per_pad), dtype=np.float32)
        xt[:, :per] = np.asarray(x[c * per : (c + 1) * per], dtype=np.float32).T
        ins.append(
            dict(
                IDX=idx_w,
                SRAW=sraw,
                DSTLOC=dst_col,
                ATTR=col(at_sl),
                TYPE=col(ty_sl),
                XT=xt,
            )
        )

    meta = dict(
        N=N, E=E, per=per, nb=nb, per_pad=per_pad, SA=SA, T=T,
        totslot=totslot, groups=groups, call_cnt=call_cnt, tiles_bh=tiles_bh,
        slot_off=slot_off, tcols_b=tcols_b,
    )
    return meta, ins


def _build(meta):
    N = meta["N"]
    per = meta["per"]
    nb = meta["nb"]
    per_pad = meta["per_pad"]
    SA = meta["SA"]
    SB = per - SA
    T = meta["T"]
    totslot = meta["totslot"]
    groups = meta["groups"]
    call_cnt = meta["call_cnt"]
    tiles_bh = meta["tiles_bh"]
    tcols_b = meta["tcols_b"]

    f32 = mybir.dt.float32
    bf16 = mybir.dt.bfloat16
    i16 = mybir.dt.int16

    maxw16 = max(c // 16 for c in call_cnt)
    maxw128 = max(c // 128 for c in call_cnt)
    call_base = [sum(call_cnt[:i]) for i in range(len(call_cnt))]

    nc = bacc.Bacc("TRN2", num_devices=N_CORES, num_swdge_queues=4,
                   dynamic_dma_scratch_size=65536)

    t_idx = nc.dram_tensor("IDX", [128, totslot // 16], i16, kind="ExternalInput")
    t_sraw = nc.dram_tensor("SRAW", [totslot, 128], bf16, kind="ExternalInput")
    t_dstloc = nc.dram_tensor("DSTLOC", [128, T], bf16, kind="ExternalInput")
    t_attr = nc.dram_tensor("ATTR", [128, T], f32, kind="ExternalInput")
    t_type = nc.dram_tensor("TYPE", [128, T], f32, kind="ExternalInput")
    t_xt = nc.dram_tensor("XT", [128, per_pad], f32, kind="ExternalInput")
    t_W = [
        nc.dram_tensor(f"W{i}", [128, 128], f32, kind="ExternalInput") for i in (1, 2, 3)
    ]
    t_b = [
        nc.dram_tensor(f"b{i}", [128, 1], f32, kind="ExternalInput") for i in (1, 2, 3)
    ]
    t_ets = nc.dram_tensor("ETS", [128, 4], f32, kind="ExternalInput")
    t_iota_b = nc.dram_tensor("IOTAB", [128, 128], bf16, kind="ExternalInput")
    t_ident = nc.dram_tensor("IDENT", [128, 128], f32, kind="ExternalInput")
    t_identb = nc.dram_tensor("IDENTB", [128, 128], bf16, kind="ExternalInput")
    t_ones_c = nc.dram_tensor("ONESC", [128, 1], bf16, kind="ExternalInput")
    t_ones_r = nc.dram_tensor("ONESR", [1, 128], f32, kind="ExternalInput")
    t_out = nc.dram_tensor("OUT", [per, 128], f32, kind="ExternalOutput")

    hcurA = [
        nc.dram_tensor(f"hcurA{l}", [SA, 128], bf16, kind="Internal") for l in range(3)
    ]
    hcurB = [
        nc.dram_tensor(f"hcurB{l}", [SB, 128], bf16, kind="Internal") for l in range(3)
    ]
    hfullA = [
        nc.dram_tensor(
            f"hfullA{l}", [N_CORES * SA, 128], bf16, kind="Internal",
            addr_space="Shared",
        )
        for l in range(3)
    ]
    hfullB = [
        nc.dram_tensor(
            f"hfullB{l}", [N_CORES * SB, 128], bf16, kind="Internal",
            addr_space="Shared",
        )
        for l in range(3)
    ]
    t_swc = nc.dram_tensor("swcache", [totslot, 128], bf16, kind="Internal")
    rg = [list(range(N_CORES))]

    def chunks512(total):
        out = []
        o = 0
        while o < total:
            w = min(512, total - o)
            out.append((o, w))
            o += w
        return out

    with TileContext(nc) as tc:
        with (
            tc.tile_pool(name="persist", bufs=1) as pp,
            tc.tile_pool(name="work", bufs=2) as wp,
            tc.tile_pool(name="swp", bufs=3) as swp,
            tc.tile_pool(name="psum", bufs=2, space="PSUM") as psp,
            tc.tile_pool(name="psumg", bufs=3, space="PSUM") as pspg,
        ):
            # ---------- persistent loads ----------
            DSTLOC = pp.tile([128, T], bf16, tag="DSTLOC")
            nc.sync.dma_start(DSTLOC[:, :], t_dstloc[:, :])
            IOTAB = pp.tile([128, 128], bf16, tag="IOTAB")
            nc.sync.dma_start(IOTAB[:, :], t_iota_b[:, :])
            IDENT = pp.tile([128, 128], f32, tag="IDENT")
            nc.sync.dma_start(IDENT[:, :], t_ident[:, :])
            IDENTB = pp.tile([128, 128], bf16, tag="IDENTB")
            nc.sync.dma_start(IDENTB[:, :], t_identb[:, :])
            ONESC = pp.tile([128, 1], bf16, tag="ONESC")
            nc.sync.dma_start(ONESC[:, :], t_ones_c[:, :])
            ONESR = pp.tile([1, 128], f32, tag="ONESR")
            nc.sync.dma_start(ONESR[:, :], t_ones_r[:, :])
            ETS = pp.tile([128, 4], f32, tag="ETS")
            nc.sync.dma_start(ETS[:, :], t_ets[:, :])
            W = []
            B = []
            for i in range(3):
                Wt = pp.tile([128, 128], f32, tag=f"W{i}")
                nc.sync.dma_start(Wt[:, :], t_W[i][:, :])
                W.append(Wt)
                Bt = pp.tile([128, 1], f32, tag=f"B{i}")
                nc.sync.dma_start(Bt[:, :], t_b[i][:, :])
                B.append(Bt)

            WCOL = pp.tile([128, T], bf16, tag="WCOL")
            DEGR = pp.tile([1, per_pad], f32, tag="DEGR")
            DINVB = pp.tile([128, per_pad], f32, tag="DINVB")
            HOUT = pp.tile([128, per_pad], f32, tag="HOUT")
            HP = pp.tile([128, per_pad], bf16, tag="HP")

            # ---------- edge weights w = scale[type]*attr (+1 for self loops) ----
            for o, cw in [(i, min(512, T - i)) for i in range(0, T, 512)]:
                at = wp.tile([128, 512], f32, tag="atc", bufs=1)
                ty = wp.tile([128, 512], f32, tag="tyc", bufs=1)
                nc.sync.dma_start(at[:, :cw], t_attr[:, o : o + cw])
                nc.sync.dma_start(ty[:, :cw], t_type[:, o : o + cw])
                acc = wp.tile([128, 512], f32, tag="acc", bufs=1)
                nc.vector.tensor_scalar(
                    acc[:, :cw], ty[:, :cw], 0.0, ETS[:, 0:1],
                    op0=mybir.AluOpType.is_equal, op1=mybir.AluOpType.mult,
                )
                for s in (1, 2, 3):
                    tmp = wp.tile([128, 512], f32, tag="wtmp", bufs=1)
                    nc.vector.tensor_scalar(
                        tmp[:, :cw], ty[:, :cw], float(s), ETS[:, s : s + 1],
                        op0=mybir.AluOpType.is_equal, op1=mybir.AluOpType.mult,
                    )
                    nc.vector.tensor_tensor(
                        acc[:, :cw], acc[:, :cw], tmp[:, :cw], op=mybir.AluOpType.add
                    )
                tmp = wp.tile([128, 512], f32, tag="wtmp", bufs=1)
                nc.vector.tensor_scalar(
                    tmp[:, :cw], ty[:, :cw], 4.0, None, op0=mybir.AluOpType.is_equal
                )
                nc.vector.tensor_tensor(
                    acc[:, :cw], acc[:, :cw], tmp[:, :cw], op=mybir.AluOpType.add
                )
                nc.vector.tensor_tensor(
                    WCOL[:, o : o + cw], acc[:, :cw], at[:, :cw],
                    op=mybir.AluOpType.mult,
                )

            def build_sw(ci):
                """S_w tiles for call ci: load host one-hot S_raw and scale by
                the device-computed edge weight (1 DVE op)."""
                cnt = call_cnt[ci]
                nt = cnt // 128
                base = call_base[ci] // 128
                sw = swp.tile([128, maxw128, 128], bf16, tag="swg")
                srv = t_sraw[call_base[ci] : call_base[ci] + cnt, :].rearrange(
                    "(t e) d -> e t d", e=128
                )
                nc.sync.dma_start(sw[:, :nt, :], srv)
                w_b = (
                    WCOL[:, base : base + nt].unsqueeze(2).to_broadcast([128, nt, 128])
                )
                nc.vector.tensor_tensor(
                    sw[:, :nt, :], sw[:, :nt, :], w_b, op=mybir.AluOpType.mult
                )
                return sw

            # ---------- degree pass (also builds + caches S_w in DRAM) ----
            for gi, g in enumerate(groups):
                gw = len(g) * 128
                sws = {}
                for h in (0, 1):
                    ci = 2 * gi + h
                    cnt = call_cnt[ci]
                    nt = cnt // 128
                    sws[h] = build_sw(ci)
                    swv = t_swc[call_base[ci] : call_base[ci] + cnt, :].rearrange(
                        "(t e) d -> e t d", e=128
                    )
                    nc.sync.dma_start(swv, sws[h][:, :nt, :])
                for bi, b in enumerate(g):
                    pd = psp.tile([1, 512], f32, tag="deg", bufs=1)
                    nlo = int(tiles_bh[b][0])
                    nhi = int(tiles_bh[b][1])
                    # 4-tile-wide deg matmuls; psum holds 4 partial deg copies
                    chunks = []
                    for hh, ntl in ((0, nlo), (1, nhi)):
                        j0 = tcols_b[b][0 if hh == 0 else nlo] - call_base[
                            2 * gi + hh
                        ] // 128
                        o = 0
                        while o < ntl:
                            wdt = min(4, ntl - o)
                            chunks.append((hh, j0 + o, wdt))
                            o += wdt
                    for k, (hh, j, wdt) in enumerate(chunks):
                        nc.tensor.matmul(
                            pd[0:1, 0 : wdt * 128],
                            ONESC[:, :], sws[hh][:, j : j + wdt, :],
                            start=(k == 0), stop=(k == len(chunks) - 1),
                        )
                    # fold the 4 partials into DEGR[b]
                    b0 = b * 128
                    nc.vector.tensor_copy(DEGR[0:1, b0 : b0 + 128], pd[0:1, 0:128])
                    for j in (1, 2, 3):
                        nc.vector.tensor_tensor(
                            DEGR[0:1, b0 : b0 + 128], DEGR[0:1, b0 : b0 + 128],
                            pd[0:1, j * 128 : (j + 1) * 128], op=mybir.AluOpType.add,
                        )
            # broadcast deg to all partitions via rank-1 outer product on PE,
            # then DINVB = 1/sqrt(deg + 1) (+1 = self loop) on full-width ops
            for o, cw in chunks512(per_pad):
                pb = psp.tile([128, 512], f32, tag="p512")
                nc.tensor.matmul(
                    pb[:, :cw], ONESR[:, :], DEGR[0:1, o : o + cw],
                    start=True, stop=True,
                )
                sq = wp.tile([128, 512], f32, tag="sq")
                nc.scalar.activation(
                    sq[:, :cw], pb[:, :cw],
                    mybir.ActivationFunctionType.Sqrt, bias=1.0, scale=1.0,
                )
                nc.vector.reciprocal(DINVB[:, o : o + cw], sq[:, :cw])

            # ---------- h1' = dinv * (x @ W1) ----------
            for o, cw in chunks512(per_pad):
                xc = wp.tile([128, 512], f32, tag="xc")
                nc.sync.dma_start(xc[:, :cw], t_xt[:, o : o + cw])
                ph = psp.tile([128, 512], f32, tag="p512")
                nc.tensor.matmul(ph[:, :cw], W[0][:, :], xc[:, :cw], start=True, stop=True)
                nc.vector.tensor_tensor(
                    HP[:, o : o + cw], ph[:, :cw], DINVB[:, o : o + cw],
                    op=mybir.AluOpType.mult,
                )

            # ---------- layers ----------
            for l in range(3):
                # HP (feat x node, bf16) -> rows (PE transpose) -> hcurA/B
                # -> two AllGathers (A fires as soon as its rows are stored)
                ag_a_done = False
                for cb in range(nb):
                    pt = psp.tile([128, 128], bf16, tag="ptr", bufs=1)
                    nc.tensor.transpose(
                        pt[:, :], HP[:, cb * 128 : (cb + 1) * 128], IDENTB[:, :]
                    )
                    rt = wp.tile([128, 128], bf16, tag="rowb")
                    nc.vector.tensor_copy(rt[:, :], pt[:, :])
                    r0 = cb * 128
                    r1 = min(per, r0 + 128)
                    if r1 <= SA:
                        nc.sync.dma_start(hcurA[l][r0:r1, :], rt[0 : r1 - r0, :])
                    elif r0 >= SA:
                        nc.sync.dma_start(
                            hcurB[l][r0 - SA : r1 - SA, :], rt[0 : r1 - r0, :]
                        )
                    else:
                        nc.sync.dma_start(hcurA[l][r0:SA, :], rt[0 : SA - r0, :])
                        nc.sync.dma_start(
                            hcurB[l][0 : r1 - SA, :], rt[SA - r0 : r1 - r0, :]
                        )
                    if r1 >= SA and not ag_a_done:
                        nc.gpsimd.collective_compute(
                            "AllGather", mybir.AluOpType.bypass,
                            ins=[hcurA[l][:, :]], outs=[hfullA[l][:, :]],
                            replica_groups=rg,
                        )
                        ag_a_done = True
                nc.gpsimd.collective_compute(
                    "AllGather", mybir.AluOpType.bypass,
                    ins=[hcurB[l][:, :]], outs=[hfullB[l][:, :]],
                    replica_groups=rg,
                )

                for gi, g in enumerate(groups):
                    gw = len(g) * 128
                    mts = {}
                    for h in (0, 1):
                        ci = 2 * gi + h
                        cnt = call_cnt[ci]
                        woff = call_base[ci] // 16
                        idxt = wp.tile([128, maxw16], i16, tag="idx", bufs=8)
                        nc.sync.dma_start(
                            idxt[:, : cnt // 16], t_idx[:, woff : woff + cnt // 16]
                        )
                        mt = wp.tile([128, maxw128, 128], bf16, tag=f"m{h}", bufs=6)
                        src_tab = hfullA[l][:, :] if h == 0 else hfullB[l][:, :]
                        nc.gpsimd.dma_gather(
                            mt[:, : cnt // 128, :], src_tab, idxt[:, : cnt // 16],
                            num_idxs=cnt, num_idxs_reg=cnt, elem_size=128,
                            single_packet=False, queue_num=ci % 4,
                        )
                        mts[h] = mt
                    sws = {}
                    for h in (0, 1):
                        ci2 = 2 * gi + h
                        cnt2 = call_cnt[ci2]
                        nt2 = cnt2 // 128
                        swl = swp.tile([128, maxw128, 128], bf16, tag="swg")
                        swv = t_swc[
                            call_base[ci2] : call_base[ci2] + cnt2, :
                        ].rearrange("(t e) d -> e t d", e=128)
                        nc.sync.dma_start(swl[:, :nt2, :], swv)
                        sws[h] = swl

                    pg = pspg.tile([128, 512], f32, tag="pblk")
                    for bi, b in enumerate(g):
                        cols = tcols_b[b]
                        nlo = int(tiles_bh[b][0])
                        for ti, tcol in enumerate(cols):
                            hh = 0 if ti < nlo else 1
                            j = tcol - call_base[2 * gi + hh] // 128
                            nc.tensor.matmul(
                                pg[:, bi * 128 : (bi + 1) * 128],
                                mts[hh][:, j, :], sws[hh][:, j, :],
                                start=(ti == 0), stop=(ti == len(cols) - 1),
                            )
                    # batched epilogue for the whole group
                    g0 = g[0] * 128
                    ep = wp.tile([128, 512], f32, tag="ep")
                    nc.vector.tensor_tensor(
                        ep[:, :gw], pg[:, :gw], HP[:, g0 : g0 + gw],
                        op=mybir.AluOpType.add,
                    )
                    nc.vector.tensor_tensor(
                        ep[:, :gw], ep[:, :gw], DINVB[:, g0 : g0 + gw],
                        op=mybir.AluOpType.mult,
                    )
                    if l < 2 and LEAKY_VIA_PRELU:
                        nc.scalar.activation(
                            HOUT[:, g0 : g0 + gw], ep[:, :gw],
                            mybir.ActivationFunctionType.Prelu,
                            bias=B[l][:, 0:1], scale=1.0, alpha=NEG_SLOPE,
                        )
                    elif l < 2:
                        t2 = wp.tile([128, 512], f32, tag="ep2")
                        nc.scalar.activation(
                            t2[:, :gw], ep[:, :gw],
                            mybir.ActivationFunctionType.Identity,
                            bias=B[l][:, 0:1], scale=1.0,
                        )
                        t3 = wp.tile([128, 512], f32, tag="ep3")
                        nc.vector.tensor_scalar_mul(t3[:, :gw], t2[:, :gw], NEG_SLOPE)
                        nc.vector.tensor_tensor(
                            HOUT[:, g0 : g0 + gw], t2[:, :gw], t3[:, :gw],
                            op=mybir.AluOpType.max,
                        )
                    else:
                        nc.scalar.activation(
                            HOUT[:, g0 : g0 + gw], ep[:, :gw],
                            mybir.ActivationFunctionType.Identity,
                            bias=B[l][:, 0:1], scale=1.0,
                        )

                if l < 2:
                    # HP = dinv * (HOUT @ W[l+1])
                    for o, cw in chunks512(per_pad):
                        ph = psp.tile([128, 512], f32, tag="p512")
                        nc.tensor.matmul(
                            ph[:, :cw], W[l + 1][:, :], HOUT[:, o : o + cw],
                            start=True, stop=True,
                        )
                        nc.vector.tensor_tensor(
                            HP[:, o : o + cw], ph[:, :cw], DINVB[:, o : o + cw],
                            op=mybir.AluOpType.mult,
                        )
                else:
                    # final: transpose HOUT (f32) to rows and store
                    for cb in range(nb):
                        pt = psp.tile([128, 128], f32, tag="ptrf", bufs=1)
                        nc.tensor.transpose(
                            pt[:, :], HOUT[:, cb * 128 : (cb + 1) * 128], IDENT[:, :]
                        )
                        rf = wp.tile([128, 128], f32, tag="rowf")
                        nc.vector.tensor_copy(rf[:, :], pt[:, :])
                        r0 = cb * 128
                        r1 = min(per, r0 + 128)
                        nc.sync.dma_start(t_out[r0:r1, :], rf[0 : r1 - r0, :])

    nc.compile()
    return nc


_CACHE = {}


def kernel(
    x,
    edge_index,
    edge_attr,
    edge_type,
    edge_type_scale,
    W1,
    b1,
    W2,
    b2,
    W3,
    b3,
):
    x = np.asarray(x)
    N = x.shape[0]
    meta, per_core = _preprocess(
        np.asarray(x), np.asarray(edge_index), np.asarray(edge_attr),
        np.asarray(edge_type),
    )

    key = (N, meta["T"], tuple(meta["call_cnt"]))
    if key not in _CACHE:
        _CACHE[key] = _build(meta)
    nc = _CACHE[key]

    ets_b = np.tile(np.asarray(edge_type_scale, np.float32)[None, :], (128, 1))
    iota_f = np.tile(np.arange(128, dtype=np.float32)[None, :], (128, 1))
    ident = np.eye(128, dtype=np.float32)
    common = dict(
        W1=np.asarray(W1, np.float32),
        W2=np.asarray(W2, np.float32),
        W3=np.asarray(W3, np.float32),
        b1=np.asarray(b1, np.float32).reshape(D, 1),
        b2=np.asarray(b2, np.float32).reshape(D, 1),
        b3=np.asarray(b3, np.float32).reshape(D, 1),
        ETS=np.ascontiguousarray(ets_b),
        IOTAB=iota_f.astype(BF16),
        IDENT=ident,
        IDENTB=ident.astype(BF16),
        ONESC=np.ones((128, 1), np.float32).astype(BF16),
        ONESR=np.ones((1, 128), np.float32),
    )
    in_maps = []
    for c in range(N_CORES):
        m = dict(common)
        m["IDX"] = per_core[c]["IDX"]
        m["SRAW"] = per_core[c]["SRAW"]
        m["DSTLOC"] = per_core[c]["DSTLOC"]
        m["ATTR"] = per_core[c]["ATTR"]
        m["TYPE"] = per_core[c]["TYPE"]
        m["XT"] = per_core[c]["XT"]
        in_maps.append(m)

    res = run_bass_kernel_spmd(
        nc, in_maps, core_ids=list(range(N_CORES)), **_RUN_KWARGS
    )
    _LAST_RESULT.clear()
    _LAST_RESULT["exec_time_ns"] = res.exec_time_ns
    _LAST_RESULT["profile_json"] = res.profile_json
    out = np.concatenate([res.results[c]["OUT"] for c in range(N_CORES)], axis=0)
    return out.astype(np.float32)


_RUN_KWARGS = {}  # test harness can set {"trace": True, "tmpdir": ...}
_LAST_RESULT = {}

